# revision 4
# baseline (speedup 1.0000x reference)
"""Chamfer distance v2: d-twice, 4-way row-tiled matmuls, split PSUM exit.

Per chunk [128 i x 1024 j] of each per-cluster distance matrix (both
orientations):
  - PE writes the chunk into a PSUM "duo" tile (2 chunks / 4 banks),
    4 row-tiled K=13 matmuls running concurrently (tile_position).
  - ACT casts the two j-half-1 segments of the duo to SBUF bf16 in one
    [128, 2, 512] instruction (PSUM exit via the scalar engine).
  - A custom DVE op (FOLD_MIN_ANT: out = min(in0,in1), accum_out =
    min-reduce) reads j-half-0 from PSUM and the bf16 half from SBUF,
    emitting the chunk's row-min in one 512-cycle pass (PSUM exit via
    DVE at 2 elems/lane/cycle aggregate).
Host sums the [128, 256] per-chunk row-min matrix, masking the top
cluster id, exactly as the reference does.
"""

import numpy as np

C = 128
P = 1024
DIM = 3
K = 13
N_CORES = 8
CPC = C // N_CORES   # 16 clusters per core
ICH = P // 128       # 8 chunks per cluster-direction
OUT_COLS = 2 * CPC * ICH  # 256

_cache = {}


def _get_fold_min_op():
    """Register (once) a custom DVE op: out = min(in0, in1);
    accum_out = min-reduce(out, init=s0)."""
    from concourse.dve_spec import Spec, Src0, Src1, C0, minn
    from concourse import dve_ops as dvo
    from concourse.dve_table_gen import dve_ver_for

    name = "FOLD_MIN_ANT"
    for op in dvo.OPS:
        if op.name == name:
            return op
    op = dvo.DveOp(
        name,
        Spec(body=minn(Src0, Src1), accum=minn, accum_init=C0),
        subdim=False,
        uops_sha={},
    )
    dvo.OPS.append(op)
    dvo.CUSTOM_DVE_SPECS[name] = op.spec
    dvo._SUB_OPCODE_FOR_NAME[name] = max(dvo._SUB_OPCODE_FOR_NAME.values()) + 1
    ver = dve_ver_for("TRN2")
    try:
        op.compile(ver)
    except ValueError as e:
        got = str(e).split(f"{ver}: ")[1].split(" ≠")[0].strip()
        op.uops_sha[ver] = got
    op.compile(ver)
    return op


def _build():
    import concourse.bacc as bacc
    import concourse.mybir as mybir
    from concourse.tile import TileContext

    fold_min = _get_fold_min_op()

    nc = bacc.Bacc(
        "TRN2", target_bir_lowering=False, debug=False, num_devices=N_CORES)
    f32 = mybir.dt.float32
    f16 = mybir.dt.float16
    bf16 = mybir.dt.bfloat16

    # weights: chunk c=g*4+w of cluster cl at partitions [32w, 32w+K),
    #          cols (cl*2+g)*128 ... +128
    aw_d = nc.dram_tensor("aw", [128, CPC * 2 * 128], f16, kind="ExternalInput")
    bw_d = nc.dram_tensor("bw", [128, CPC * 2 * 128], f16, kind="ExternalInput")
    # rhs replicated at the 4 partition offsets: cluster cl at cols cl*1024
    ar_d = nc.dram_tensor("ar", [128, CPC * P], f16, kind="ExternalInput")
    br_d = nc.dram_tensor("br", [128, CPC * P], f16, kind="ExternalInput")
    out_d = nc.dram_tensor("out", [128, OUT_COLS], f32, kind="ExternalOutput")

    with TileContext(nc) as tc:
        with (
            tc.tile_pool(name="io", bufs=1) as iop,
            tc.tile_pool(name="psum", bufs=2, space="PSUM") as pp,
            tc.tile_pool(name="scr", bufs=4) as sp,
        ):
            aw_t = iop.tile([128, CPC * 2 * 128], f16)
            bw_t = iop.tile([128, CPC * 2 * 128], f16)
            ar_t = iop.tile([128, CPC * P], f16)
            br_t = iop.tile([128, CPC * P], f16)
            # first slice small (1 cluster) so compute starts ASAP, then
            # the rest in growing pieces; dir-0 operands (aw/br) first
            bounds = [0, 1, 3, 6, 10, CPC]
            for q in range(len(bounds) - 1):
                ws = slice(bounds[q] * 2 * 128, bounds[q + 1] * 2 * 128)
                rs = slice(bounds[q] * P, bounds[q + 1] * P)
                nc.sync.dma_start(out=aw_t[:, ws], in_=aw_d[:, ws])
                nc.sync.dma_start(out=br_t[:, rs], in_=br_d[:, rs])
            for q in range(len(bounds) - 1):
                ws = slice(bounds[q] * 2 * 128, bounds[q + 1] * 2 * 128)
                rs = slice(bounds[q] * P, bounds[q + 1] * P)
                nc.sync.dma_start(out=bw_t[:, ws], in_=bw_d[:, ws])
                nc.sync.dma_start(out=ar_t[:, rs], in_=ar_d[:, rs])
            mins_t = iop.tile([128, OUT_COLS], f32)
            # persistent 8-bank PSUM tile; bank h*4+w holds (chunk w, jhalf h)
            ps = pp.tile([128, 8, 512], f32, name="ps", bufs=1)

            for dirn in range(2):
                wt, rt = (aw_t, br_t) if dirn == 0 else (bw_t, ar_t)
                for cl in range(CPC):
                    for g in range(2):
                        # h1 first: its banks are freed by the (early) ACT
                        # cast of the previous wave, so PE never stalls on
                        # the custom-op drain of banks 0-3.
                        for h in (1, 0):
                            for w in range(4):
                                lhsT = wt[32 * w:32 * w + K,
                                          (cl * 2 + g) * 128:
                                          (cl * 2 + g + 1) * 128]
                                rhs = rt[32 * w:32 * w + K,
                                         cl * P + h * 512:cl * P + h * 512 + 512]
                                nc.tensor.matmul(
                                    ps[:, h * 4 + w, :],
                                    lhsT, rhs, start=True, stop=True,
                                    tile_position=(32 * w, 0))
                        e_w = sp.tile([128, 4, 512], bf16, tag="e1",
                                      bufs=5, name="e_w")
                        # two half-casts: customs c0/c1 unblock ~800ns
                        # earlier than with a single 4-bank cast
                        nc.scalar.copy(out=e_w[:, 0:2, :], in_=ps[:, 4:6, :])
                        nc.scalar.copy(out=e_w[:, 2:4, :], in_=ps[:, 6:8, :])
                        for w in range(4):
                            c = g * 4 + w
                            col = (dirn * CPC + cl) * ICH + c
                            scb = sp.tile([128, 512], bf16, tag="scb",
                                          name="scb")
                            nc.vector._custom_dve(
                                fold_min, out=scb[:],
                                in0=ps[:, w, :],
                                in1=e_w[:, w, :],
                                s0=3.0e38,
                                accum_out=mins_t[:, col:col + 1])

            nc.sync.dma_start(out=out_d[:], in_=mins_t[:])
    nc.compile()
    return nc


def _split(x):
    hi = x.astype(np.float16)
    lo = (x - hi.astype(np.float32)).astype(np.float16)
    return hi, lo


def _prep(input_points, output_points):
    a = np.ascontiguousarray(input_points, dtype=np.float32).reshape(C, P, DIM)
    b = np.ascontiguousarray(output_points, dtype=np.float32).reshape(C, P, DIM)
    aa = np.einsum("cpd,cpd->cp", a, a).astype(np.float32)
    bb = np.einsum("cpd,cpd->cp", b, b).astype(np.float32)

    at = a.transpose(0, 2, 1)            # [C,3,P]
    bt2 = -2.0 * b.transpose(0, 2, 1)    # [C,3,P]
    ah, al = _split(at)
    bh, bl = _split(bt2)
    aah, aal = _split(aa)
    bbh, bbl = _split(bb)

    # d = sum_k A[k,i] * B[k,j]
    a_op = np.empty((C, K, P), np.float16)
    a_op[:, 0:3] = ah
    a_op[:, 3:6] = al
    a_op[:, 6:9] = ah
    a_op[:, 9:11] = 1.0
    a_op[:, 11] = aah
    a_op[:, 12] = aal

    b_op = np.empty((C, K, P), np.float16)
    b_op[:, 0:3] = bh
    b_op[:, 3:6] = bh
    b_op[:, 6:9] = bl
    b_op[:, 9] = bbh
    b_op[:, 10] = bbl
    b_op[:, 11:13] = 1.0

    def weights_layout(op_sl):
        # op_sl: [CPC, K, P] -> [128, CPC*2*128]
        # chunk c = g*4+w at partitions 32w..32w+K, cols (cl*2+g)*128
        v = op_sl.reshape(CPC, K, 2, 4, 128)     # cl, k, g, w, x
        out = np.zeros((4, 32, CPC, 2, 128), np.float16)
        out[:, :K] = v.transpose(3, 1, 0, 2, 4)  # w, k, cl, g, x
        return out.reshape(128, CPC * 2 * 128)

    def rhs_layout(op_sl):
        # op_sl: [CPC, K, P] -> [128, CPC*P] replicated at 4 offsets
        out = np.zeros((4, 32, CPC, P), np.float16)
        out[:, :K] = op_sl.transpose(1, 0, 2)[None]
        return out.reshape(128, CPC * P)

    in_maps = []
    for i in range(N_CORES):
        sl = slice(i * CPC, (i + 1) * CPC)
        in_maps.append({
            "aw": weights_layout(a_op[sl]),
            "bw": weights_layout(b_op[sl]),
            "ar": rhs_layout(a_op[sl]),
            "br": rhs_layout(b_op[sl]),
        })
    return in_maps


def run(inputs, trace=False, trace_kwargs=None):
    from concourse.bass_utils import run_bass_kernel_spmd

    if "nc" not in _cache:
        _cache["nc"] = _build()
    nc = _cache["nc"]

    in_maps = _prep(inputs["input_points"], inputs["output_points"])
    res = run_bass_kernel_spmd(
        nc, in_maps, list(range(N_CORES)),
        trace=trace, **(trace_kwargs or {}))

    per_cluster = np.concatenate([
        res.results[i]["out"].reshape(128, 2, CPC, ICH).sum(
            axis=(0, 1, 3), dtype=np.float64)
        for i in range(N_CORES)
    ])  # [C]

    nb = int(np.max(inputs["input_clusters"]))
    mask = np.arange(C) < nb
    total = np.float32(per_cluster[mask].sum())
    return np.array(total, dtype=np.float32), res


def kernel(input_points, input_clusters, output_points, output_clusters):
    loss, _ = run({
        "input_points": input_points,
        "input_clusters": input_clusters,
        "output_points": output_points,
        "output_clusters": output_clusters,
    })
    return loss


# revision 5
# speedup vs baseline: 1.0121x; 1.0121x over previous
"""Chamfer distance v2: d-twice, 4-way row-tiled matmuls, split PSUM exit.

Per chunk [128 i x 1024 j] of each per-cluster distance matrix (both
orientations):
  - PE writes the chunk into a PSUM "duo" tile (2 chunks / 4 banks),
    4 row-tiled K=13 matmuls running concurrently (tile_position).
  - ACT casts the two j-half-1 segments of the duo to SBUF bf16 in one
    [128, 2, 512] instruction (PSUM exit via the scalar engine).
  - A custom DVE op (FOLD_MIN_ANT: out = min(in0,in1), accum_out =
    min-reduce) reads j-half-0 from PSUM and the bf16 half from SBUF,
    emitting the chunk's row-min in one 512-cycle pass (PSUM exit via
    DVE at 2 elems/lane/cycle aggregate).
Host sums the [128, 256] per-chunk row-min matrix, masking the top
cluster id, exactly as the reference does.
"""

import numpy as np

C = 128
P = 1024
DIM = 3
K = 13
N_CORES = 8
CPC = C // N_CORES   # 16 clusters per core
ICH = P // 128       # 8 chunks per cluster-direction
OUT_COLS = 2 * CPC * ICH  # 256

_cache = {}


def _get_fold_min_op():
    """Register (once) a custom DVE op: out = min(in0, in1);
    accum_out = min-reduce(out, init=s0)."""
    from concourse.dve_spec import Spec, Src0, Src1, C0, minn
    from concourse import dve_ops as dvo
    from concourse.dve_table_gen import dve_ver_for

    name = "FOLD_MIN_ANT"
    for op in dvo.OPS:
        if op.name == name:
            return op
    op = dvo.DveOp(
        name,
        Spec(body=minn(Src0, Src1), accum=minn, accum_init=C0),
        subdim=False,
        uops_sha={},
    )
    dvo.OPS.append(op)
    dvo.CUSTOM_DVE_SPECS[name] = op.spec
    dvo._SUB_OPCODE_FOR_NAME[name] = max(dvo._SUB_OPCODE_FOR_NAME.values()) + 1
    ver = dve_ver_for("TRN2")
    try:
        op.compile(ver)
    except ValueError as e:
        got = str(e).split(f"{ver}: ")[1].split(" ≠")[0].strip()
        op.uops_sha[ver] = got
    op.compile(ver)
    return op


def _build():
    import concourse.bacc as bacc
    import concourse.mybir as mybir
    from concourse.tile import TileContext

    fold_min = _get_fold_min_op()

    nc = bacc.Bacc(
        "TRN2", target_bir_lowering=False, debug=False, num_devices=N_CORES)
    f32 = mybir.dt.float32
    f16 = mybir.dt.float16
    bf16 = mybir.dt.bfloat16

    # weights: chunk c=g*4+w of cluster cl at partitions [32w, 32w+K),
    #          cols (cl*2+g)*128 ... +128
    aw_d = nc.dram_tensor("aw", [128, CPC * 2 * 128], f16, kind="ExternalInput")
    bw_d = nc.dram_tensor("bw", [128, CPC * 2 * 128], f16, kind="ExternalInput")
    # rhs replicated at the 4 partition offsets: cluster cl at cols cl*1024
    ar_d = nc.dram_tensor("ar", [128, CPC * P], f16, kind="ExternalInput")
    br_d = nc.dram_tensor("br", [128, CPC * P], f16, kind="ExternalInput")
    out_d = nc.dram_tensor("out", [128, OUT_COLS], f32, kind="ExternalOutput")

    with TileContext(nc) as tc:
        with (
            tc.tile_pool(name="io", bufs=1) as iop,
            tc.tile_pool(name="psum", bufs=2, space="PSUM") as pp,
            tc.tile_pool(name="scr", bufs=4) as sp,
        ):
            aw_t = iop.tile([128, CPC * 2 * 128], f16)
            bw_t = iop.tile([128, CPC * 2 * 128], f16)
            ar_t = iop.tile([128, CPC * P], f16)
            br_t = iop.tile([128, CPC * P], f16)
            # first slice small (1 cluster) so compute starts ASAP, then
            # the rest in growing pieces; dir-0 operands (aw/br) first
            bounds = [0, 1, 3, 6, 10, CPC]
            for q in range(len(bounds) - 1):
                ws = slice(bounds[q] * 2 * 128, bounds[q + 1] * 2 * 128)
                rs = slice(bounds[q] * P, bounds[q + 1] * P)
                nc.sync.dma_start(out=aw_t[:, ws], in_=aw_d[:, ws])
                nc.sync.dma_start(out=br_t[:, rs], in_=br_d[:, rs])
            for q in range(len(bounds) - 1):
                ws = slice(bounds[q] * 2 * 128, bounds[q + 1] * 2 * 128)
                rs = slice(bounds[q] * P, bounds[q + 1] * P)
                nc.sync.dma_start(out=bw_t[:, ws], in_=bw_d[:, ws])
                nc.sync.dma_start(out=ar_t[:, rs], in_=ar_d[:, rs])
            mins_t = iop.tile([128, OUT_COLS], f32)
            # persistent 8-bank PSUM tile; bank h*4+w holds (chunk w, jhalf h)
            ps = pp.tile([128, 8, 512], f32, name="ps", bufs=1)

            for dirn in range(2):
                wt, rt = (aw_t, br_t) if dirn == 0 else (bw_t, ar_t)
                for cl in range(CPC):
                    for g in range(2):
                        # h1 first: its banks are freed by the (early) ACT
                        # cast of the previous wave, so PE never stalls on
                        # the custom-op drain of banks 0-3.
                        for h in (1, 0):
                            for w in range(4):
                                lhsT = wt[32 * w:32 * w + K,
                                          (cl * 2 + g) * 128:
                                          (cl * 2 + g + 1) * 128]
                                rhs = rt[32 * w:32 * w + K,
                                         cl * P + h * 512:cl * P + h * 512 + 512]
                                nc.tensor.matmul(
                                    ps[:, h * 4 + w, :],
                                    lhsT, rhs, start=True, stop=True,
                                    tile_position=(32 * w, 0))
                        e_w = sp.tile([128, 4, 512], bf16, tag="e1",
                                      bufs=3, name="e_w")
                        # two half-casts: customs c0/c1 unblock ~800ns
                        # earlier than with a single 4-bank cast
                        nc.scalar.copy(out=e_w[:, 0:2, :], in_=ps[:, 4:6, :])
                        nc.scalar.copy(out=e_w[:, 2:4, :], in_=ps[:, 6:8, :])
                        for w in range(4):
                            c = g * 4 + w
                            col = (dirn * CPC + cl) * ICH + c
                            scb = sp.tile([128, 512], bf16, tag="scb",
                                          name="scb")
                            nc.vector._custom_dve(
                                fold_min, out=scb[:],
                                in0=ps[:, w, :],
                                in1=e_w[:, w, :],
                                s0=3.0e38,
                                accum_out=mins_t[:, col:col + 1])

            nc.sync.dma_start(out=out_d[:], in_=mins_t[:])
    nc.compile()
    return nc


def _split(x):
    hi = x.astype(np.float16)
    lo = (x - hi.astype(np.float32)).astype(np.float16)
    return hi, lo


def _prep(input_points, output_points):
    a = np.ascontiguousarray(input_points, dtype=np.float32).reshape(C, P, DIM)
    b = np.ascontiguousarray(output_points, dtype=np.float32).reshape(C, P, DIM)
    aa = np.einsum("cpd,cpd->cp", a, a).astype(np.float32)
    bb = np.einsum("cpd,cpd->cp", b, b).astype(np.float32)

    at = a.transpose(0, 2, 1)            # [C,3,P]
    bt2 = -2.0 * b.transpose(0, 2, 1)    # [C,3,P]
    ah, al = _split(at)
    bh, bl = _split(bt2)
    aah, aal = _split(aa)
    bbh, bbl = _split(bb)

    # d = sum_k A[k,i] * B[k,j]
    a_op = np.empty((C, K, P), np.float16)
    a_op[:, 0:3] = ah
    a_op[:, 3:6] = al
    a_op[:, 6:9] = ah
    a_op[:, 9:11] = 1.0
    a_op[:, 11] = aah
    a_op[:, 12] = aal

    b_op = np.empty((C, K, P), np.float16)
    b_op[:, 0:3] = bh
    b_op[:, 3:6] = bh
    b_op[:, 6:9] = bl
    b_op[:, 9] = bbh
    b_op[:, 10] = bbl
    b_op[:, 11:13] = 1.0

    def weights_layout(op_sl):
        # op_sl: [CPC, K, P] -> [128, CPC*2*128]
        # chunk c = g*4+w at partitions 32w..32w+K, cols (cl*2+g)*128
        v = op_sl.reshape(CPC, K, 2, 4, 128)     # cl, k, g, w, x
        out = np.zeros((4, 32, CPC, 2, 128), np.float16)
        out[:, :K] = v.transpose(3, 1, 0, 2, 4)  # w, k, cl, g, x
        return out.reshape(128, CPC * 2 * 128)

    def rhs_layout(op_sl):
        # op_sl: [CPC, K, P] -> [128, CPC*P] replicated at 4 offsets
        out = np.zeros((4, 32, CPC, P), np.float16)
        out[:, :K] = op_sl.transpose(1, 0, 2)[None]
        return out.reshape(128, CPC * P)

    in_maps = []
    for i in range(N_CORES):
        sl = slice(i * CPC, (i + 1) * CPC)
        in_maps.append({
            "aw": weights_layout(a_op[sl]),
            "bw": weights_layout(b_op[sl]),
            "ar": rhs_layout(a_op[sl]),
            "br": rhs_layout(b_op[sl]),
        })
    return in_maps


def run(inputs, trace=False, trace_kwargs=None):
    from concourse.bass_utils import run_bass_kernel_spmd

    if "nc" not in _cache:
        _cache["nc"] = _build()
    nc = _cache["nc"]

    in_maps = _prep(inputs["input_points"], inputs["output_points"])
    res = run_bass_kernel_spmd(
        nc, in_maps, list(range(N_CORES)),
        trace=trace, **(trace_kwargs or {}))

    per_cluster = np.concatenate([
        res.results[i]["out"].reshape(128, 2, CPC, ICH).sum(
            axis=(0, 1, 3), dtype=np.float64)
        for i in range(N_CORES)
    ])  # [C]

    nb = int(np.max(inputs["input_clusters"]))
    mask = np.arange(C) < nb
    total = np.float32(per_cluster[mask].sum())
    return np.array(total, dtype=np.float32), res


def kernel(input_points, input_clusters, output_points, output_clusters):
    loss, _ = run({
        "input_points": input_points,
        "input_clusters": input_clusters,
        "output_points": output_points,
        "output_clusters": output_clusters,
    })
    return loss


# revision 6
# speedup vs baseline: 1.0152x; 1.0031x over previous
"""Chamfer distance v2: d-twice, 4-way row-tiled matmuls, split PSUM exit.

Per chunk [128 i x 1024 j] of each per-cluster distance matrix (both
orientations):
  - PE writes the chunk into a PSUM "duo" tile (2 chunks / 4 banks),
    4 row-tiled K=13 matmuls running concurrently (tile_position).
  - ACT casts the two j-half-1 segments of the duo to SBUF bf16 in one
    [128, 2, 512] instruction (PSUM exit via the scalar engine).
  - A custom DVE op (FOLD_MIN_ANT: out = min(in0,in1), accum_out =
    min-reduce) reads j-half-0 from PSUM and the bf16 half from SBUF,
    emitting the chunk's row-min in one 512-cycle pass (PSUM exit via
    DVE at 2 elems/lane/cycle aggregate).
Host sums the [128, 256] per-chunk row-min matrix, masking the top
cluster id, exactly as the reference does.
"""

import numpy as np

C = 128
P = 1024
DIM = 3
K = 13
N_CORES = 8
CPC = C // N_CORES   # 16 clusters per core
ICH = P // 128       # 8 chunks per cluster-direction
OUT_COLS = 2 * CPC * ICH  # 256

_cache = {}


def _get_fold_min_op():
    """Register (once) a custom DVE op: out = min(in0, in1);
    accum_out = min-reduce(out, init=s0)."""
    from concourse.dve_spec import Spec, Src0, Src1, C0, minn
    from concourse import dve_ops as dvo
    from concourse.dve_table_gen import dve_ver_for

    name = "FOLD_MIN_ANT"
    for op in dvo.OPS:
        if op.name == name:
            return op
    op = dvo.DveOp(
        name,
        Spec(body=minn(Src0, Src1), accum=minn, accum_init=C0),
        subdim=False,
        uops_sha={},
    )
    dvo.OPS.append(op)
    dvo.CUSTOM_DVE_SPECS[name] = op.spec
    dvo._SUB_OPCODE_FOR_NAME[name] = max(dvo._SUB_OPCODE_FOR_NAME.values()) + 1
    ver = dve_ver_for("TRN2")
    try:
        op.compile(ver)
    except ValueError as e:
        got = str(e).split(f"{ver}: ")[1].split(" ≠")[0].strip()
        op.uops_sha[ver] = got
    op.compile(ver)
    return op


def _build():
    import concourse.bacc as bacc
    import concourse.mybir as mybir
    from concourse.tile import TileContext

    fold_min = _get_fold_min_op()

    nc = bacc.Bacc(
        "TRN2", target_bir_lowering=False, debug=False, num_devices=N_CORES)
    f32 = mybir.dt.float32
    f16 = mybir.dt.float16
    bf16 = mybir.dt.bfloat16

    # weights: chunk c=g*4+w of cluster cl at partitions [32w, 32w+K),
    #          cols (cl*2+g)*128 ... +128
    aw_d = nc.dram_tensor("aw", [128, CPC * 2 * 128], f16, kind="ExternalInput")
    bw_d = nc.dram_tensor("bw", [128, CPC * 2 * 128], f16, kind="ExternalInput")
    # rhs replicated at the 4 partition offsets: cluster cl at cols cl*1024
    ar_d = nc.dram_tensor("ar", [128, CPC * P], f16, kind="ExternalInput")
    br_d = nc.dram_tensor("br", [128, CPC * P], f16, kind="ExternalInput")
    out_d = nc.dram_tensor("out", [128, OUT_COLS], f32, kind="ExternalOutput")

    with TileContext(nc) as tc:
        with (
            tc.tile_pool(name="io", bufs=1) as iop,
            tc.tile_pool(name="psum", bufs=2, space="PSUM") as pp,
            tc.tile_pool(name="scr", bufs=4) as sp,
        ):
            aw_t = iop.tile([128, CPC * 2 * 128], f16)
            bw_t = iop.tile([128, CPC * 2 * 128], f16)
            ar_t = iop.tile([128, CPC * P], f16)
            br_t = iop.tile([128, CPC * P], f16)
            # first slice small (1 cluster) so compute starts ASAP, then
            # the rest in growing pieces; dir-0 operands (aw/br) first
            bounds = [0, 1, 3, 6, 10, CPC]
            for q in range(len(bounds) - 1):
                ws = slice(bounds[q] * 2 * 128, bounds[q + 1] * 2 * 128)
                rs = slice(bounds[q] * P, bounds[q + 1] * P)
                nc.sync.dma_start(out=aw_t[:, ws], in_=aw_d[:, ws])
                if q == 0:
                    nc.sync.dma_start(out=br_t[:, 512:1024],
                                      in_=br_d[:, 512:1024])
                    nc.sync.dma_start(out=br_t[:, 0:512], in_=br_d[:, 0:512])
                else:
                    nc.sync.dma_start(out=br_t[:, rs], in_=br_d[:, rs])
            for q in range(len(bounds) - 1):
                ws = slice(bounds[q] * 2 * 128, bounds[q + 1] * 2 * 128)
                rs = slice(bounds[q] * P, bounds[q + 1] * P)
                nc.sync.dma_start(out=bw_t[:, ws], in_=bw_d[:, ws])
                nc.sync.dma_start(out=ar_t[:, rs], in_=ar_d[:, rs])
            mins_t = iop.tile([128, OUT_COLS], f32)
            # persistent 8-bank PSUM tile; bank h*4+w holds (chunk w, jhalf h)
            ps = pp.tile([128, 8, 512], f32, name="ps", bufs=1)

            for dirn in range(2):
                if dirn == 1:
                    nc.sync.dma_start(out=out_d[:, 0:128],
                                      in_=mins_t[:, 0:128])
                wt, rt = (aw_t, br_t) if dirn == 0 else (bw_t, ar_t)
                for cl in range(CPC):
                    for g in range(2):
                        # h1 first: its banks are freed by the (early) ACT
                        # cast of the previous wave, so PE never stalls on
                        # the custom-op drain of banks 0-3.
                        for h in (1, 0):
                            for w in range(4):
                                lhsT = wt[32 * w:32 * w + K,
                                          (cl * 2 + g) * 128:
                                          (cl * 2 + g + 1) * 128]
                                rhs = rt[32 * w:32 * w + K,
                                         cl * P + h * 512:cl * P + h * 512 + 512]
                                nc.tensor.matmul(
                                    ps[:, h * 4 + w, :],
                                    lhsT, rhs, start=True, stop=True,
                                    tile_position=(32 * w, 0))
                        e_w = sp.tile([128, 4, 512], bf16, tag="e1",
                                      bufs=3, name="e_w")
                        # two half-casts: customs c0/c1 unblock ~800ns
                        # earlier than with a single 4-bank cast
                        nc.scalar.copy(out=e_w[:, 0:2, :], in_=ps[:, 4:6, :])
                        nc.scalar.copy(out=e_w[:, 2:4, :], in_=ps[:, 6:8, :])
                        for w in range(4):
                            c = g * 4 + w
                            col = (dirn * CPC + cl) * ICH + c
                            scb = sp.tile([128, 512], bf16, tag="scb",
                                          name="scb")
                            nc.vector._custom_dve(
                                fold_min, out=scb[:],
                                in0=ps[:, w, :],
                                in1=e_w[:, w, :],
                                s0=3.0e38,
                                accum_out=mins_t[:, col:col + 1])

            nc.sync.dma_start(out=out_d[:, 128:256],
                              in_=mins_t[:, 128:256])
    nc.compile()
    return nc


def _split(x):
    hi = x.astype(np.float16)
    lo = (x - hi.astype(np.float32)).astype(np.float16)
    return hi, lo


def _prep(input_points, output_points):
    a = np.ascontiguousarray(input_points, dtype=np.float32).reshape(C, P, DIM)
    b = np.ascontiguousarray(output_points, dtype=np.float32).reshape(C, P, DIM)
    aa = np.einsum("cpd,cpd->cp", a, a).astype(np.float32)
    bb = np.einsum("cpd,cpd->cp", b, b).astype(np.float32)

    at = a.transpose(0, 2, 1)            # [C,3,P]
    bt2 = -2.0 * b.transpose(0, 2, 1)    # [C,3,P]
    ah, al = _split(at)
    bh, bl = _split(bt2)
    aah, aal = _split(aa)
    bbh, bbl = _split(bb)

    # d = sum_k A[k,i] * B[k,j]
    a_op = np.empty((C, K, P), np.float16)
    a_op[:, 0:3] = ah
    a_op[:, 3:6] = al
    a_op[:, 6:9] = ah
    a_op[:, 9:11] = 1.0
    a_op[:, 11] = aah
    a_op[:, 12] = aal

    b_op = np.empty((C, K, P), np.float16)
    b_op[:, 0:3] = bh
    b_op[:, 3:6] = bh
    b_op[:, 6:9] = bl
    b_op[:, 9] = bbh
    b_op[:, 10] = bbl
    b_op[:, 11:13] = 1.0

    def weights_layout(op_sl):
        # op_sl: [CPC, K, P] -> [128, CPC*2*128]
        # chunk c = g*4+w at partitions 32w..32w+K, cols (cl*2+g)*128
        v = op_sl.reshape(CPC, K, 2, 4, 128)     # cl, k, g, w, x
        out = np.zeros((4, 32, CPC, 2, 128), np.float16)
        out[:, :K] = v.transpose(3, 1, 0, 2, 4)  # w, k, cl, g, x
        return out.reshape(128, CPC * 2 * 128)

    def rhs_layout(op_sl):
        # op_sl: [CPC, K, P] -> [128, CPC*P] replicated at 4 offsets
        out = np.zeros((4, 32, CPC, P), np.float16)
        out[:, :K] = op_sl.transpose(1, 0, 2)[None]
        return out.reshape(128, CPC * P)

    in_maps = []
    for i in range(N_CORES):
        sl = slice(i * CPC, (i + 1) * CPC)
        in_maps.append({
            "aw": weights_layout(a_op[sl]),
            "bw": weights_layout(b_op[sl]),
            "ar": rhs_layout(a_op[sl]),
            "br": rhs_layout(b_op[sl]),
        })
    return in_maps


def run(inputs, trace=False, trace_kwargs=None):
    from concourse.bass_utils import run_bass_kernel_spmd

    if "nc" not in _cache:
        _cache["nc"] = _build()
    nc = _cache["nc"]

    in_maps = _prep(inputs["input_points"], inputs["output_points"])
    res = run_bass_kernel_spmd(
        nc, in_maps, list(range(N_CORES)),
        trace=trace, **(trace_kwargs or {}))

    per_cluster = np.concatenate([
        res.results[i]["out"].reshape(128, 2, CPC, ICH).sum(
            axis=(0, 1, 3), dtype=np.float64)
        for i in range(N_CORES)
    ])  # [C]

    nb = int(np.max(inputs["input_clusters"]))
    mask = np.arange(C) < nb
    total = np.float32(per_cluster[mask].sum())
    return np.array(total, dtype=np.float32), res


def kernel(input_points, input_clusters, output_points, output_clusters):
    loss, _ = run({
        "input_points": input_points,
        "input_clusters": input_clusters,
        "output_points": output_points,
        "output_clusters": output_clusters,
    })
    return loss
